# revision 20
# baseline (speedup 1.0000x reference)
"""Trainium2 Bass kernel for GQA attention (B=2, S=2048, D=2048, 16 q-heads /
4 kv-heads, HD=128) with per-head QK RMSNorm + RoPE + causal softmax + output
projection.

Sharding: 8 cores = (batch b in {0,1}) x (kv-group g in {0..3}). Each core
computes its batch's 4 q-heads + 1 kv-head and a partial output through the
row-sharded Wo; the host sums the 4 partials per batch.
"""
import numpy as np

import concourse.bass as bass  # noqa: F401
import concourse.mybir as mybir
import concourse.tile as tile
from concourse import bacc
from concourse.bass_utils import run_bass_kernel_spmd

F32 = mybir.dt.float32
F32R = mybir.dt.float32r
F16 = mybir.dt.float16
AF = mybir.ActivationFunctionType
OP = mybir.AluOpType

B, S, D = 2, 2048, 2048
NH, NKV, HD = 16, 4, 128
REP = NH // NKV
EPS = 1e-6
NEG = -1.0e30
EXPB = -5.0  # exp bias: cancels in softmax, keeps exp() in fp16 range


def build(s=S):
    """Build + compile the per-core SPMD program (identical on all 8 cores)."""
    sc = s // 128          # s-chunks
    kc = D // 128          # contraction chunks
    nsb = sc // 4          # q superblocks (512 wide)
    nc = bacc.Bacc("TRN2", target_bir_lowering=False, debug=False, num_devices=8)

    xT_d = nc.dram_tensor("xT", [D, s], F16, kind="ExternalInput")
    wqkv_d = nc.dram_tensor("wqkv", [D, 768], F16, kind="ExternalInput")
    wo_d = nc.dram_tensor("wo", [512, D], F16, kind="ExternalInput")
    cwq_d = nc.dram_tensor("cwq", [s, HD], F32, kind="ExternalInput")
    swq_d = nc.dram_tensor("swq", [s, HD], F32, kind="ExternalInput")
    cwk_d = nc.dram_tensor("cwk", [s, HD], F32, kind="ExternalInput")
    swk_d = nc.dram_tensor("swk", [s, HD], F32, kind="ExternalInput")
    mask_d = nc.dram_tensor("maskb", [128, 128], F32, kind="ExternalInput")
    iden16_d = nc.dram_tensor("ident16", [128, 128], F16, kind="ExternalInput")
    out_d = nc.dram_tensor("outp", [s, D], F32, kind="ExternalOutput")

    with tile.TileContext(nc) as tc:
        with (
            tc.tile_pool(name="pers", bufs=1) as pers,
            tc.tile_pool(name="psA", bufs=2, space="PSUM") as psA,   # [128,1024]
            tc.tile_pool(name="psB", bufs=2, space="PSUM") as psB,   # [128,512]
            tc.tile_pool(name="psT", bufs=2, space="PSUM") as psT,   # [128,512]
        ):
            qT = pers.tile([128, REP, s], F16, tag="qT")
            kT = pers.tile([128, s], F16, tag="kT")
            vv = pers.tile([128, sc, HD], F16, tag="vv")
            aoT = pers.tile([128, REP, s], F16, tag="aoT")
            mask_t = pers.tile([128, 128], F32, tag="maskb")
            iden16_t = pers.tile([128, 128], F16, tag="ident16")
            nc.sync.dma_start(out=mask_t[:], in_=mask_d[:, :])
            nc.sync.dma_start(out=iden16_t[:], in_=iden16_d[:, :])
            eps_t = pers.tile([128, 1], F32, tag="eps")
            nc.vector.memset(eps_t[:], EPS)
            expb_t = pers.tile([128, 1], F32, tag="expb")
            nc.vector.memset(expb_t[:], EXPB)
            zero_t = pers.tile([128, 384], F16, tag="zeros")
            nc.vector.memset(zero_t[:], 0.0)
            ones_t = pers.tile([128, 128], F16, tag="ones")
            nc.vector.memset(ones_t[:], 1.0)

            # ---------------- Phase 1: QKV + RMSNorm + RoPE -----------------
            with (
                tc.tile_pool(name="wq", bufs=1) as wq,
                tc.tile_pool(name="xp", bufs=3) as xp,
                tc.tile_pool(name="cp", bufs=3) as cp,
                tc.tile_pool(name="st", bufs=3) as st,
            ):
                wqkv_t = wq.tile([128, kc, 768], F16, tag="wqkv")
                wqkv_r = wqkv_d.rearrange("(dk ki) e -> ki dk e", ki=128)
                cwq_r = cwq_d.rearrange("(m si) h -> si m h", si=128)
                swq_r = swq_d.rearrange("(m si) h -> si m h", si=128)
                cwk_r = cwk_d.rearrange("(m si) h -> si m h", si=128)
                swk_r = swk_d.rearrange("(m si) h -> si m h", si=128)
                xT_r = xT_d.rearrange("(dk ki) t -> ki dk t", ki=128)

                for k in range(kc):
                    nc.sync.dma_start(out=wqkv_t[:, k], in_=wqkv_r[:, k])

                for m in range(sc):
                    xt = xp.tile([128, kc, 128], F16, tag="xt")
                    nc.gpsimd.dma_start(out=xt[:], in_=xT_r[:, :, m * 128:(m + 1) * 128])
                    cq = cp.tile([128, 128], F32, tag="cq")
                    sq_ = cp.tile([128, 128], F32, tag="sq_")
                    ck = cp.tile([128, 128], F32, tag="ck")
                    sk_ = cp.tile([128, 128], F32, tag="sk_")
                    nc.gpsimd.dma_start(out=cq[:], in_=cwq_r[:, m])
                    nc.gpsimd.dma_start(out=sq_[:], in_=swq_r[:, m])
                    nc.gpsimd.dma_start(out=ck[:], in_=cwk_r[:, m])
                    nc.gpsimd.dma_start(out=sk_[:], in_=swk_r[:, m])

                    pqt = psA.tile([128, 1024], F32, tag="psA")
                    pq = pqt[:, 0:512]
                    pkv = psB.tile([128, 512], F32, tag="psB")
                    for k in range(kc):
                        nc.tensor.matmul(
                            pq, xt[:, k], wqkv_t[:, k, 0:512],
                            start=(k == 0), stop=(k == kc - 1),
                        )
                    for k in range(kc):
                        nc.tensor.matmul(
                            pkv[:, 0:256], xt[:, k], wqkv_t[:, k, 512:768],
                            start=(k == 0), stop=(k == kc - 1),
                        )

                    # ---- batched RMSNorm stats: one Square per q block ----
                    ss = st.tile([128, 16], F32, tag="ss")
                    sqs = st.tile([128, 512], F32, tag="sqs")
                    nc.scalar.activation(sqs[:], pq, AF.Square)
                    sqk = st.tile([128, 128], F32, tag="sqk")
                    nc.scalar.activation(
                        sqk[:], pkv[:, 0:128], AF.Square, accum_out=ss[:, 4:5],
                    )
                    nc.vector.tensor_reduce(
                        out=ss[:, 0:4],
                        in_=sqs[:].rearrange("p (h d) -> p h d", d=128),
                        axis=mybir.AxisListType.X, op=OP.add,
                    )
                    nc.scalar.activation(
                        ss[:, 8:13], ss[:, 0:5], AF.Sqrt,
                        bias=eps_t[:], scale=1.0 / HD,
                    )
                    rs = st.tile([128, 8], F32, tag="rs")
                    nc.vector.reciprocal(rs[:, 0:5], ss[:, 8:13])

                    # ---- fused RoPE for all 4 q heads (broadcast APs) ----
                    pq3 = pq.rearrange("p (h d) -> p h d", d=128)
                    u = st.tile([128, REP, 128], F32, tag="u")
                    nc.vector.tensor_mul(
                        u[:], pq3,
                        rs[:, 0:4].rearrange("p (h o) -> p h o", o=1).broadcast_to(
                            [128, REP, 128]),
                    )
                    qn = st.tile([128, 512], F16, tag="qn")
                    qn3 = qn[:].rearrange("p (h d) -> p h d", d=128)
                    ra = st.tile([128, REP, 128], F32, tag="ra")
                    nc.vector.tensor_mul(
                        ra[:], u[:],
                        cq[:].rearrange("p (o d) -> p o d", o=1).broadcast_to(
                            [128, REP, 128]),
                    )
                    rb = st.tile([128, REP, 128], F32, tag="rb")
                    nc.vector.tensor_mul(
                        rb[:, :, 0:64], u[:, :, 64:128],
                        sq_[:, 0:64].rearrange("p (o d) -> p o d", o=1).broadcast_to(
                            [128, REP, 64]),
                    )
                    nc.vector.tensor_mul(
                        rb[:, :, 64:128], u[:, :, 0:64],
                        sq_[:, 64:128].rearrange("p (o d) -> p o d", o=1).broadcast_to(
                            [128, REP, 64]),
                    )
                    nc.vector.tensor_add(qn3, ra[:], rb[:])

                    # ---- k head rope ----
                    uk = st.tile([128, 128], F32, tag="uk")
                    nc.vector.tensor_scalar_mul(uk[:], pkv[:, 0:128], rs[:, 4:5])
                    kn = st.tile([128, 128], F16, tag="kn")
                    rak = st.tile([128, 128], F32, tag="rak")
                    nc.vector.tensor_mul(rak[:], uk[:], ck[:])
                    rbk = st.tile([128, 128], F32, tag="rbk")
                    nc.vector.tensor_mul(rbk[:, 0:64], uk[:, 64:128], sk_[:, 0:64])
                    nc.vector.tensor_mul(rbk[:, 64:128], uk[:, 0:64], sk_[:, 64:128])
                    nc.vector.tensor_add(kn[:], rak[:], rbk[:])

                    # ---- transposes to head-major ----
                    for h in range(REP):
                        pt = psT.tile([128, 512], F16, tag="psT")
                        nc.tensor.transpose(
                            pt[:, 0:128], qn[:, h * 128:(h + 1) * 128], iden16_t[:],
                        )
                        nc.vector.tensor_copy(
                            out=qT[:, h, m * 128:(m + 1) * 128], in_=pt[:, 0:128],
                        )
                    pt = psT.tile([128, 512], F16, tag="psT")
                    nc.tensor.transpose(pt[:, 0:128], kn[:], iden16_t[:])
                    nc.vector.tensor_copy(
                        out=kT[:, m * 128:(m + 1) * 128], in_=pt[:, 0:128],
                    )
                    nc.vector.tensor_copy(out=vv[:, m, :], in_=pkv[:, 128:256])

            # ------- Phase 2+3: causal attention + fused out-projection -----
            with (
                tc.tile_pool(name="wop", bufs=1) as wop,
                tc.tile_pool(name="pp", bufs=2) as pp,
                tc.tile_pool(name="tb", bufs=3) as tb,
                tc.tile_pool(name="ap", bufs=4) as ap,
                tc.tile_pool(name="ob", bufs=2) as ob,
            ):
                wo_t = wop.tile([128, REP, D], F16, tag="wo")
                nc.sync.dma_start(
                    out=wo_t[:], in_=wo_d.rearrange("(e ki) d -> ki e d", ki=128),
                )
                for Q in range(nsb):
                    for h in range(REP):
                        nj = 4 * Q + 4
                        sums = psT.tile([128, 512], F32, tag="psT")
                        pvp = psB.tile([128, 512], F32, tag="psB")
                        for jp in range(0, nj, 2):
                            js = [j for j in (jp, jp + 1) if j < nj]
                            ps_s = psA.tile([128, 1024], F32, tag="psA")
                            pT = pp.tile([128, 1024], F16, tag="probsT")
                            regions = []
                            for j in js:
                                off = (j - jp) * 512
                                c0 = (j - 4 * Q) * 128 if j > 4 * Q else 0
                                nc.tensor.matmul(
                                    ps_s[:, off + c0:off + 512],
                                    kT[:, j * 128:(j + 1) * 128],
                                    qT[:, h, Q * 512 + c0:(Q + 1) * 512],
                                    start=True, stop=True,
                                )
                                if j >= 4 * Q:
                                    dc = (j - 4 * Q) * 128
                                    nc.vector.tensor_add(
                                        ps_s[:, off + dc:off + dc + 128],
                                        ps_s[:, off + dc:off + dc + 128],
                                        mask_t[:],
                                    )
                                regions.append((off + c0, off + 512))
                            # merge contiguous exp regions across the pair
                            merged = []
                            for a, b in regions:
                                if merged and merged[-1][1] == a:
                                    merged[-1][1] = b
                                else:
                                    merged.append([a, b])
                            for a, b in merged:
                                nc.scalar.activation(
                                    pT[:, a:b], ps_s[:, a:b], AF.Exp,
                                    bias=expb_t[:],
                                )
                            for j in js:
                                off = (j - jp) * 512
                                c0 = (j - 4 * Q) * 128 if j > 4 * Q else 0
                                nc.tensor.matmul(
                                    sums[0:1, c0:512], ones_t[:, 0:1],
                                    pT[:, off + c0:off + 512],
                                    start=(j == 0), stop=(j == nj - 1),
                                    skip_group_check=True,
                                )
                                nc.tensor.matmul(
                                    pvp[:, c0:512], vv[:, j, :],
                                    pT[:, off + c0:off + 512],
                                    start=(j == 0), stop=(j == nj - 1),
                                    skip_group_check=True,
                                )
                        rec = tb.tile([128, 512], F16, tag="rec")
                        with nc.allow_low_precision(reason="softmax recip to fp16"):
                            nc.vector.reciprocal(rec[0:1, :], sums[0:1, :])
                        bcp = psT.tile([128, 512], F32, tag="psT")
                        nc.tensor.matmul(
                            bcp[:], ones_t[0:1, :], rec[0:1, :],
                            start=True, stop=True,
                        )
                        bcs = tb.tile([128, 512], F32, tag="bcs")
                        nc.vector.tensor_copy(out=bcs[:], in_=bcp[:])
                        nc.vector.tensor_mul(
                            aoT[:, h, Q * 512:(Q + 1) * 512], pvp[:], bcs[:],
                        )
                    # fused out-projection for this superblock's s-chunks
                    for m in range(4 * Q, 4 * Q + 4):
                        ot = ob.tile([128, D], F32, tag="ot")
                        for n in range(D // 512):
                            po = psB.tile([128, 512], F32, tag="psB")
                            for e in range(REP):
                                nc.tensor.matmul(
                                    po[:], aoT[:, e, m * 128:(m + 1) * 128],
                                    wo_t[:, e, n * 512:(n + 1) * 512],
                                    start=(e == 0), stop=(e == REP - 1),
                                )
                            if n % 2 == 0:
                                nc.vector.tensor_copy(
                                    out=ot[:, n * 512:(n + 1) * 512], in_=po[:],
                                )
                            else:
                                nc.scalar.copy(
                                    out=ot[:, n * 512:(n + 1) * 512], in_=po[:],
                                )
                        nc.sync.dma_start(
                            out=out_d[m * 128:(m + 1) * 128, :], in_=ot[:],
                        )

    nc.compile()
    return nc


def make_in_maps(x, cos, sin, Wq, Wk, Wv, Wo, q_norm_w, k_norm_w):
    qsc = (q_norm_w / np.sqrt(HD)).astype(np.float32)
    ksc = k_norm_w.astype(np.float32)

    def rope_consts(w):
        cw = (cos * w[None, :]).astype(np.float32)
        sw = np.empty_like(cw)
        sw[:, :64] = -sin[:, :64] * w[None, 64:]
        sw[:, 64:] = sin[:, 64:] * w[None, :64]
        return cw, sw

    cwq, swq = rope_consts(qsc)
    cwk, swk = rope_consts(ksc)
    r = np.arange(128)
    maskb = np.where(r[:, None] > r[None, :], NEG, 0.0).astype(np.float32)
    ident16 = np.eye(128, dtype=np.float16)

    in_maps = []
    for c in range(8):
        b, g = c // 4, c % 4
        xT = np.ascontiguousarray(x[b].T.astype(np.float16))
        wqkv = np.ascontiguousarray(
            np.concatenate(
                [
                    Wq[:, g * 512:(g + 1) * 512],
                    Wk[:, g * 128:(g + 1) * 128],
                    Wv[:, g * 128:(g + 1) * 128],
                ],
                axis=1,
            ).astype(np.float16)
        )
        wo = np.ascontiguousarray(Wo[g * 512:(g + 1) * 512, :].astype(np.float16))
        in_maps.append(
            dict(
                xT=xT, wqkv=wqkv, wo=wo, cwq=cwq, swq=swq, cwk=cwk, swk=swk,
                maskb=maskb, ident16=ident16,
            )
        )
    return in_maps


_cached = None


def kernel(x, cos, sin, Wq, Wk, Wv, Wo, q_norm_w, k_norm_w):
    global _cached
    x = np.asarray(x, np.float32)
    cos = np.asarray(cos, np.float32)
    sin = np.asarray(sin, np.float32)
    in_maps = make_in_maps(
        x, cos, sin,
        np.asarray(Wq, np.float32), np.asarray(Wk, np.float32),
        np.asarray(Wv, np.float32), np.asarray(Wo, np.float32),
        np.asarray(q_norm_w, np.float32), np.asarray(k_norm_w, np.float32),
    )
    if _cached is None:
        _cached = build()
    res = run_bass_kernel_spmd(_cached, in_maps, core_ids=list(range(8)))
    out = np.zeros((B, S, D), np.float64)
    for c in range(8):
        out[c // 4] += res.results[c]["outp"].astype(np.float64)
    return out.astype(np.float32)


# revision 24
# speedup vs baseline: 1.2316x; 1.2316x over previous
"""Trainium2 Bass kernel for GQA attention (B=2, S=2048, D=2048, 16 q-heads /
4 kv-heads, HD=128) with per-head QK RMSNorm + RoPE + causal softmax + output
projection.

Sharding: 8 cores = (batch b in {0,1}) x (kv-group g in {0..3}). Each core
computes its batch's 4 q-heads + 1 kv-head and a partial output through the
row-sharded Wo; the host sums the 4 partials per batch.
"""
import numpy as np

import concourse.bass as bass  # noqa: F401
import concourse.mybir as mybir
import concourse.tile as tile
from concourse import bacc
from concourse.bass_utils import run_bass_kernel_spmd

F32 = mybir.dt.float32
F32R = mybir.dt.float32r
F16 = mybir.dt.float16
AF = mybir.ActivationFunctionType
OP = mybir.AluOpType

B, S, D = 2, 2048, 2048
NH, NKV, HD = 16, 4, 128
REP = NH // NKV
EPS = 1e-6
NEG = -1.0e30
EXPB = -5.0  # exp bias: cancels in softmax, keeps exp() in fp16 range


def build(s=S):
    """Build + compile the per-core SPMD program (identical on all 8 cores)."""
    sc = s // 128          # s-chunks
    kc = D // 128          # contraction chunks
    nsb = sc // 4          # q superblocks (512 wide)
    nc = bacc.Bacc("TRN2", target_bir_lowering=False, debug=False, num_devices=8)

    xT_d = nc.dram_tensor("xT", [D, s], F16, kind="ExternalInput")
    wqkv_d = nc.dram_tensor("wqkv", [D, 768], F16, kind="ExternalInput")
    wo_d = nc.dram_tensor("wo", [512, D], F16, kind="ExternalInput")
    cwq_d = nc.dram_tensor("cwq", [s, HD], F32, kind="ExternalInput")
    swq_d = nc.dram_tensor("swq", [s, HD], F32, kind="ExternalInput")
    cwk_d = nc.dram_tensor("cwk", [s, HD], F32, kind="ExternalInput")
    swk_d = nc.dram_tensor("swk", [s, HD], F32, kind="ExternalInput")
    mask_d = nc.dram_tensor("maskb", [128, 128], F32, kind="ExternalInput")
    iden16_d = nc.dram_tensor("ident16", [128, 128], F16, kind="ExternalInput")
    out_d = nc.dram_tensor("outp", [s, D], F32, kind="ExternalOutput")

    with tile.TileContext(nc) as tc:
        with (
            tc.tile_pool(name="pers", bufs=1) as pers,
            tc.tile_pool(name="psA", bufs=2, space="PSUM") as psA,   # [128,1024]
            tc.tile_pool(name="psB", bufs=2, space="PSUM") as psB,   # [128,512]
            tc.tile_pool(name="psT", bufs=2, space="PSUM") as psT,   # [128,512]
        ):
            qT = pers.tile([128, REP, s], F16, tag="qT")
            kT = pers.tile([128, s], F16, tag="kT")
            vv = pers.tile([128, sc, HD], F16, tag="vv")
            aoT = pers.tile([128, REP, s], F16, tag="aoT")
            mask_t = pers.tile([128, 128], F32, tag="maskb")
            iden16_t = pers.tile([128, 128], F16, tag="ident16")
            nc.sync.dma_start(out=mask_t[:], in_=mask_d[:, :])
            nc.sync.dma_start(out=iden16_t[:], in_=iden16_d[:, :])
            eps_t = pers.tile([128, 1], F32, tag="eps")
            nc.vector.memset(eps_t[:], EPS)
            expb_t = pers.tile([128, 1], F32, tag="expb")
            nc.vector.memset(expb_t[:], EXPB)
            zero_t = pers.tile([128, 384], F16, tag="zeros")
            nc.vector.memset(zero_t[:], 0.0)
            ones_t = pers.tile([128, 128], F16, tag="ones")
            nc.vector.memset(ones_t[:], 1.0)

            # ---------------- Phase 1: QKV + RMSNorm + RoPE -----------------
            with (
                tc.tile_pool(name="wq", bufs=1) as wq,
                tc.tile_pool(name="xp", bufs=3) as xp,
                tc.tile_pool(name="cp", bufs=3) as cp,
                tc.tile_pool(name="st", bufs=3) as st,
            ):
                wqkv_t = wq.tile([128, kc, 768], F16, tag="wqkv")
                wqkv_r = wqkv_d.rearrange("(dk ki) e -> ki dk e", ki=128)
                cwq_r = cwq_d.rearrange("(m si) h -> si m h", si=128)
                swq_r = swq_d.rearrange("(m si) h -> si m h", si=128)
                cwk_r = cwk_d.rearrange("(m si) h -> si m h", si=128)
                swk_r = swk_d.rearrange("(m si) h -> si m h", si=128)
                xT_r = xT_d.rearrange("(dk ki) t -> ki dk t", ki=128)

                for k in range(kc):
                    nc.sync.dma_start(out=wqkv_t[:, k], in_=wqkv_r[:, k])

                for m in range(sc):
                    xt = xp.tile([128, kc, 128], F16, tag="xt")
                    nc.gpsimd.dma_start(out=xt[:], in_=xT_r[:, :, m * 128:(m + 1) * 128])
                    cq = cp.tile([128, 128], F32, tag="cq")
                    sq_ = cp.tile([128, 128], F32, tag="sq_")
                    ck = cp.tile([128, 128], F32, tag="ck")
                    sk_ = cp.tile([128, 128], F32, tag="sk_")
                    nc.gpsimd.dma_start(out=cq[:], in_=cwq_r[:, m])
                    nc.gpsimd.dma_start(out=sq_[:], in_=swq_r[:, m])
                    nc.gpsimd.dma_start(out=ck[:], in_=cwk_r[:, m])
                    nc.gpsimd.dma_start(out=sk_[:], in_=swk_r[:, m])

                    pqt = psA.tile([128, 1024], F32, tag="psA")
                    pq = pqt[:, 0:512]
                    pkv = psB.tile([128, 512], F32, tag="psB")
                    for k in range(kc):
                        nc.tensor.matmul(
                            pq, xt[:, k], wqkv_t[:, k, 0:512],
                            start=(k == 0), stop=(k == kc - 1),
                        )
                    for k in range(kc):
                        nc.tensor.matmul(
                            pkv[:, 0:256], xt[:, k], wqkv_t[:, k, 512:768],
                            start=(k == 0), stop=(k == kc - 1),
                        )

                    # ---- batched RMSNorm stats: one Square per q block ----
                    ss = st.tile([128, 16], F32, tag="ss")
                    sqs = st.tile([128, 512], F32, tag="sqs")
                    nc.scalar.activation(sqs[:], pq, AF.Square)
                    sqk = st.tile([128, 128], F32, tag="sqk")
                    nc.scalar.activation(
                        sqk[:], pkv[:, 0:128], AF.Square, accum_out=ss[:, 4:5],
                    )
                    nc.vector.tensor_reduce(
                        out=ss[:, 0:4],
                        in_=sqs[:].rearrange("p (h d) -> p h d", d=128),
                        axis=mybir.AxisListType.X, op=OP.add,
                    )
                    nc.scalar.activation(
                        ss[:, 8:13], ss[:, 0:5], AF.Sqrt,
                        bias=eps_t[:], scale=1.0 / HD,
                    )
                    rs = st.tile([128, 8], F32, tag="rs")
                    nc.vector.reciprocal(rs[:, 0:5], ss[:, 8:13])

                    # ---- fused RoPE for all 4 q heads (broadcast APs) ----
                    pq3 = pq.rearrange("p (h d) -> p h d", d=128)
                    u = st.tile([128, REP, 128], F32, tag="u")
                    nc.vector.tensor_mul(
                        u[:], pq3,
                        rs[:, 0:4].rearrange("p (h o) -> p h o", o=1).broadcast_to(
                            [128, REP, 128]),
                    )
                    qn = st.tile([128, 512], F16, tag="qn")
                    qn3 = qn[:].rearrange("p (h d) -> p h d", d=128)
                    ra = st.tile([128, REP, 128], F32, tag="ra")
                    nc.vector.tensor_mul(
                        ra[:], u[:],
                        cq[:].rearrange("p (o d) -> p o d", o=1).broadcast_to(
                            [128, REP, 128]),
                    )
                    rb = st.tile([128, REP, 128], F32, tag="rb")
                    nc.vector.tensor_mul(
                        rb[:, :, 0:64], u[:, :, 64:128],
                        sq_[:, 0:64].rearrange("p (o d) -> p o d", o=1).broadcast_to(
                            [128, REP, 64]),
                    )
                    nc.vector.tensor_mul(
                        rb[:, :, 64:128], u[:, :, 0:64],
                        sq_[:, 64:128].rearrange("p (o d) -> p o d", o=1).broadcast_to(
                            [128, REP, 64]),
                    )
                    nc.vector.tensor_add(qn3, ra[:], rb[:])

                    # ---- k head rope ----
                    uk = st.tile([128, 128], F32, tag="uk")
                    nc.vector.tensor_scalar_mul(uk[:], pkv[:, 0:128], rs[:, 4:5])
                    kn = st.tile([128, 128], F16, tag="kn")
                    rak = st.tile([128, 128], F32, tag="rak")
                    nc.vector.tensor_mul(rak[:], uk[:], ck[:])
                    rbk = st.tile([128, 128], F32, tag="rbk")
                    nc.vector.tensor_mul(rbk[:, 0:64], uk[:, 64:128], sk_[:, 0:64])
                    nc.vector.tensor_mul(rbk[:, 64:128], uk[:, 0:64], sk_[:, 64:128])
                    nc.vector.tensor_add(kn[:], rak[:], rbk[:])

                    # ---- transposes to head-major ----
                    for h in range(REP):
                        pt = psT.tile([128, 512], F16, tag="psT")
                        nc.tensor.transpose(
                            pt[:, 0:128], qn[:, h * 128:(h + 1) * 128], iden16_t[:],
                        )
                        nc.vector.tensor_copy(
                            out=qT[:, h, m * 128:(m + 1) * 128], in_=pt[:, 0:128],
                        )
                    pt = psT.tile([128, 512], F16, tag="psT")
                    nc.tensor.transpose(pt[:, 0:128], kn[:], iden16_t[:])
                    nc.vector.tensor_copy(
                        out=kT[:, m * 128:(m + 1) * 128], in_=pt[:, 0:128],
                    )
                    nc.vector.tensor_copy(out=vv[:, m, :], in_=pkv[:, 128:256])

            # ------- Phase 2+3: causal attention + fused out-projection -----
            with (
                tc.tile_pool(name="wop", bufs=1) as wop,
                tc.tile_pool(name="pp", bufs=3) as pp,
                tc.tile_pool(name="tb", bufs=3) as tb,
                tc.tile_pool(name="ap", bufs=4) as ap,
                tc.tile_pool(name="ob", bufs=2) as ob,
            ):
                wo_t = wop.tile([128, REP, D], F16, tag="wo")
                nc.sync.dma_start(
                    out=wo_t[:], in_=wo_d.rearrange("(e ki) d -> ki e d", ki=128),
                )
                for Q in range(nsb):
                    for h in range(REP):
                        nj = 4 * Q + 4
                        sums = psT.tile([128, 512], F32, tag="psT")
                        hp = 0
                        pvp = psB.tile([128, 512], F32, tag="psB")
                        for jp in range(0, nj, 2):
                            js = [j for j in (jp, jp + 1) if j < nj]
                            ps_s = psA.tile([128, 1024], F32, tag="psA")
                            pT = pp.tile([128, 1024], F16, tag="probsT")
                            regions = []
                            for j in js:
                                off = (j - jp) * 512
                                c0 = (j - 4 * Q) * 128 if j > 4 * Q else 0
                                nc.tensor.matmul(
                                    ps_s[:, off + c0:off + 512],
                                    kT[:, j * 128:(j + 1) * 128],
                                    qT[:, h, Q * 512 + c0:(Q + 1) * 512],
                                    start=True, stop=True,
                                )
                                if j >= 4 * Q:
                                    dc = (j - 4 * Q) * 128
                                    nc.vector.tensor_add(
                                        ps_s[:, off + dc:off + dc + 128],
                                        ps_s[:, off + dc:off + dc + 128],
                                        mask_t[:],
                                    )
                                regions.append((off + c0, off + 512))
                            # merge contiguous exp regions across the pair
                            merged = []
                            for a, b in regions:
                                if merged and merged[-1][1] == a:
                                    merged[-1][1] = b
                                else:
                                    merged.append([a, b])
                            for a, b in merged:
                                nc.scalar.activation(
                                    pT[:, a:b], ps_s[:, a:b], AF.Exp,
                                    bias=expb_t[:],
                                )
                            for j in js:
                                off = (j - jp) * 512
                                c0 = (j - 4 * Q) * 128 if j > 4 * Q else 0
                                nc.tensor.matmul(
                                    sums[hp:hp + 1, c0:512], ones_t[:, 0:1],
                                    pT[:, off + c0:off + 512],
                                    start=(j == 0), stop=(j == nj - 1),
                                    skip_group_check=True,
                                )
                                nc.tensor.matmul(
                                    pvp[:, c0:512], vv[:, j, :],
                                    pT[:, off + c0:off + 512],
                                    start=(j == 0), stop=(j == nj - 1),
                                    skip_group_check=True,
                                )
                        recQ = tb.tile([128, 512], F32, tag="rec")
                        rec16 = tb.tile([128, 512], F16, tag="rec16")
                        nc.vector.reciprocal_approx_fast(
                            out=recQ[hp:hp + 1, :], in_=sums[hp:hp + 1, :],
                        )
                        with nc.allow_low_precision(reason="softmax recip fp16"):
                            nc.vector.tensor_copy(
                                out=rec16[hp:hp + 1, :], in_=recQ[hp:hp + 1, :],
                            )
                        bcp = psB.tile([128, 512], F32, tag="psB")
                        nc.tensor.matmul(
                            bcp[:], ones_t[hp:hp + 1, :], rec16[hp:hp + 1, :],
                            start=True, stop=True,
                        )
                        bcs = tb.tile([128, 512], F32, tag="bcs")
                        nc.vector.tensor_copy(out=bcs[:], in_=bcp[:])
                        nc.vector.tensor_mul(
                            aoT[:, h, Q * 512:(Q + 1) * 512], pvp[:], bcs[:],
                        )
                    # fused out-projection for this superblock's s-chunks
                    for m in range(4 * Q, 4 * Q + 4):
                        ot = ob.tile([128, D], F32, tag="ot")
                        for n in range(D // 512):
                            po = psB.tile([128, 512], F32, tag="psB")
                            for e in range(REP):
                                nc.tensor.matmul(
                                    po[:], aoT[:, e, m * 128:(m + 1) * 128],
                                    wo_t[:, e, n * 512:(n + 1) * 512],
                                    start=(e == 0), stop=(e == REP - 1),
                                )
                            if n % 2 == 0:
                                nc.vector.tensor_copy(
                                    out=ot[:, n * 512:(n + 1) * 512], in_=po[:],
                                )
                            else:
                                nc.scalar.copy(
                                    out=ot[:, n * 512:(n + 1) * 512], in_=po[:],
                                )
                        nc.sync.dma_start(
                            out=out_d[m * 128:(m + 1) * 128, :], in_=ot[:],
                        )

    nc.compile()
    return nc


def make_in_maps(x, cos, sin, Wq, Wk, Wv, Wo, q_norm_w, k_norm_w):
    qsc = (q_norm_w / np.sqrt(HD)).astype(np.float32)
    ksc = k_norm_w.astype(np.float32)

    def rope_consts(w):
        cw = (cos * w[None, :]).astype(np.float32)
        sw = np.empty_like(cw)
        sw[:, :64] = -sin[:, :64] * w[None, 64:]
        sw[:, 64:] = sin[:, 64:] * w[None, :64]
        return cw, sw

    cwq, swq = rope_consts(qsc)
    cwk, swk = rope_consts(ksc)
    r = np.arange(128)
    maskb = np.where(r[:, None] > r[None, :], NEG, 0.0).astype(np.float32)
    ident16 = np.eye(128, dtype=np.float16)

    in_maps = []
    for c in range(8):
        b, g = c // 4, c % 4
        xT = np.ascontiguousarray(x[b].T.astype(np.float16))
        wqkv = np.ascontiguousarray(
            np.concatenate(
                [
                    Wq[:, g * 512:(g + 1) * 512],
                    Wk[:, g * 128:(g + 1) * 128],
                    Wv[:, g * 128:(g + 1) * 128],
                ],
                axis=1,
            ).astype(np.float16)
        )
        wo = np.ascontiguousarray(Wo[g * 512:(g + 1) * 512, :].astype(np.float16))
        in_maps.append(
            dict(
                xT=xT, wqkv=wqkv, wo=wo, cwq=cwq, swq=swq, cwk=cwk, swk=swk,
                maskb=maskb, ident16=ident16,
            )
        )
    return in_maps


_cached = None


def kernel(x, cos, sin, Wq, Wk, Wv, Wo, q_norm_w, k_norm_w):
    global _cached
    x = np.asarray(x, np.float32)
    cos = np.asarray(cos, np.float32)
    sin = np.asarray(sin, np.float32)
    in_maps = make_in_maps(
        x, cos, sin,
        np.asarray(Wq, np.float32), np.asarray(Wk, np.float32),
        np.asarray(Wv, np.float32), np.asarray(Wo, np.float32),
        np.asarray(q_norm_w, np.float32), np.asarray(k_norm_w, np.float32),
    )
    if _cached is None:
        _cached = build()
    res = run_bass_kernel_spmd(_cached, in_maps, core_ids=list(range(8)))
    out = np.zeros((B, S, D), np.float64)
    for c in range(8):
        out[c // 4] += res.results[c]["outp"].astype(np.float64)
    return out.astype(np.float32)
